# revision 2
# baseline (speedup 1.0000x reference)
"""GCN message-passing + FFN kernel for Trainium2 (8 NeuronCores).

Strategy (dst-sharded, zero collectives), v2 fp16 datapath:
  - Sort edges by dst on host, pad nodes to 50176 = 8*49*128.
  - Core c owns dst rows [c*6272, (c+1)*6272): it processes every edge whose
    dst lands in its range, so partial aggregates never cross cores.
  - x table stored fp16 in HBM; per-edge rows gathered with dma_gather
    (256B descriptors). Gathers are GROUPED: one lo + one hi gather per
    GG-block group (int16 indices force a lo/hi table split at 32768),
    slashing SWDGE fixed overhead, and issued one group ahead (prefetch).
  - Per dst-block of 128 nodes: build a scaled one-hot mask
    [edge, dst_local] = se[e] * (dstl[e]==q) in fp16 with one fused
    tensor_scalar(is_equal, mult), spread across DVE/ACT/Pool engines, and
    matmul-accumulate aggT[feat, dst] in PSUM fp32 over the block's chunks
    (fp16 matmul = 1 cyc/row vs fp32's 4).
  - se = ew/sqrt(deg_src*deg_dst) precomputed on host (index-class prep,
    like the degree tables the packing already builds).
  - FFN fused per block in fp16: hT = relu(W1.T @ aggT + b1),
    out = hT.T @ W2 (+b2), fp32 output DMA'd per 128-row block.
"""
import sys

sys.path.insert(0, "/opt/trn_rl_repo")

import numpy as np

import concourse.bacc as bacc
import concourse.mybir as mybir
import concourse.tile as tile
from concourse.bass_utils import run_bass_kernel_spmd

P = 128
D = 128
NCORES = 8
N_NODES = 50000
NPAD = 50176          # next multiple of 128*8 above 50000
NBLK = NPAD // P      # 392 blocks
NBC = NBLK // NCORES  # 49 blocks per core
HALF = 32768          # int16 index limit for dma_gather
GG = 5                # blocks per gather group

f32 = mybir.dt.float32
f16 = mybir.dt.float16
i16 = mybir.dt.int16


def _groups():
    """Partition the 49 per-core blocks into gather groups."""
    out = []
    b = 0
    while b < NBC:
        n = min(GG, NBC - b)
        out.append(list(range(b, b + n)))
        b += n
    return out


def _host_pack(x, src, dst, edge_weights):
    """Host prep: degree tables + per-edge coefficient, dst-sort, lo/hi
    split by src, group-major columnar edge metadata + wrapped int16
    gather indices."""
    E = src.shape[0]
    src = np.asarray(src).astype(np.int64)
    dst = np.asarray(dst).astype(np.int64)
    ew = np.asarray(edge_weights).astype(np.float32)

    deg = np.bincount(src, minlength=NPAD)
    deg = np.maximum(deg, 1).astype(np.float32)  # exact ints, <=2^24
    se_e = (ew / np.sqrt(deg[src] * deg[dst])).astype(np.float32)

    order = np.argsort(dst, kind="stable")
    ds = dst[order]
    ss = src[order]
    es = se_e[order]

    g = ds >> 7                                   # global block id (dst-sorted)
    ishi = (ss >= HALF).astype(np.int64)
    key = g * 2 + ishi                            # lo edges first within block
    order2 = np.argsort(key, kind="stable")
    ds, ss, es, g, ishi, key = (a[order2] for a in (ds, ss, es, g, ishi, key))

    kcounts = np.bincount(key, minlength=NBLK * 2)
    lo_cnt = kcounts[0::2].reshape(NCORES, NBC)   # [core, slot]
    hi_cnt = kcounts[1::2].reshape(NCORES, NBC)
    # per-slot chunk counts = max over cores (SPMD single program)
    C_lo_s = np.maximum(1, np.ceil(lo_cnt.max(axis=0) / P).astype(int))
    C_hi_s = np.maximum(1, np.ceil(hi_cnt.max(axis=0) / P).astype(int))

    # group-major column order: per group, all lo chunks (block-major),
    # then all hi chunks (block-major)
    groups = _groups()
    col_lo = np.zeros(NBC, int)
    col_hi = np.zeros(NBC, int)
    loc_lo = np.zeros(NBC, int)   # chunk offset within the group's xg_lo tile
    loc_hi = np.zeros(NBC, int)
    grp_col0 = []
    c = 0
    for grp in groups:
        grp_col0.append(c)
        loc = 0
        for b in grp:
            col_lo[b] = c
            loc_lo[b] = loc
            c += C_lo_s[b]
            loc += C_lo_s[b]
        loc = 0
        for b in grp:
            col_hi[b] = c
            loc_hi[b] = loc
            c += C_hi_s[b]
            loc += C_hi_s[b]
    M = int(c)

    kstarts = np.concatenate([[0], np.cumsum(kcounts)[:-1]])
    rank = np.arange(E) - kstarts[key]            # rank within (block, lo/hi)
    b_loc = g % NBC
    core = g // NBC
    p_lane = rank % P
    t_chunk = rank // P
    col = np.where(ishi == 0, col_lo[b_loc], col_hi[b_loc]) + t_chunk

    dstl_all = np.zeros((NCORES, P, M), np.float32)
    se_all = np.zeros((NCORES, P, M), np.float32)
    dstl_all[core, p_lane, col] = (ds & 127).astype(np.float32)
    se_all[core, p_lane, col] = es

    # wrapped int16 gather index arrays: per (block-slot, half), gather slot i
    # lives at [i % 16, off + i//16]; replicated across the 8 Q7 groups.
    def build_idx(nchunk_s, sel, values):
        off16 = np.concatenate([[0], np.cumsum(nchunk_s * 8)[:-1]])
        ST = int((nchunk_s * 8).sum())            # int16 cols total
        arr = np.zeros((NCORES, 16, ST), np.int16)
        sl = rank[sel]
        cb, bb, vv = core[sel], b_loc[sel], values[sel]
        arr[cb, sl % 16, off16[bb] + sl // 16] = vv.astype(np.int16)
        return np.tile(arr, (1, 8, 1)), off16, ST

    is_lo = ishi == 0
    ilo16, lo_off16, ST_lo = build_idx(C_lo_s, is_lo, ss)
    ihi16, hi_off16, ST_hi = build_idx(C_hi_s, ~is_lo, ss - HALF)

    xpad = np.zeros((NPAD, D), np.float16)
    xpad[:N_NODES] = np.asarray(x, dtype=np.float16)
    layout = dict(C_lo_s=C_lo_s.tolist(), C_hi_s=C_hi_s.tolist(),
                  col_lo=col_lo.tolist(), col_hi=col_hi.tolist(),
                  loc_lo=loc_lo.tolist(), loc_hi=loc_hi.tolist(),
                  M=M, lo_off16=lo_off16.tolist(), hi_off16=hi_off16.tolist(),
                  ST_lo=ST_lo, ST_hi=ST_hi)
    return layout, xpad, ilo16, ihi16, dstl_all, se_all


def _build_program(layout, b2_nonzero, repeats=1, act_k=7, pool_k=9):
    """act_k / pool_k: every act_k-th (pool_k-th) chunk's mask is built on
    the ACT (Pool) engine instead of DVE, to spread mask-building load.
    0 disables the offload."""
    C_lo_s, C_hi_s = layout["C_lo_s"], layout["C_hi_s"]
    col_lo, col_hi = layout["col_lo"], layout["col_hi"]
    loc_lo, loc_hi = layout["loc_lo"], layout["loc_hi"]
    M = layout["M"]
    lo_off16, hi_off16 = layout["lo_off16"], layout["hi_off16"]
    ST_lo, ST_hi = layout["ST_lo"], layout["ST_hi"]
    groups = _groups()
    nc = bacc.Bacc("TRN2", target_bir_lowering=False, debug=False,
                   num_swdge_queues=3)

    xt = nc.dram_tensor("xt", [NPAD, D], f16, kind="ExternalInput")
    ilo_d = nc.dram_tensor("ilo", [P, ST_lo], i16, kind="ExternalInput")
    ihi_d = nc.dram_tensor("ihi", [P, ST_hi], i16, kind="ExternalInput")
    dstl_d = nc.dram_tensor("dstl", [P, M], f32, kind="ExternalInput")
    se_d = nc.dram_tensor("se", [P, M], f32, kind="ExternalInput")
    iota_d = nc.dram_tensor("iota", [P, P], f16, kind="ExternalInput")
    w1_d = nc.dram_tensor("w1", [D, D], f16, kind="ExternalInput")
    w2_d = nc.dram_tensor("w2", [D, D], f16, kind="ExternalInput")
    b1_d = nc.dram_tensor("b1", [D, 1], f32, kind="ExternalInput")
    if b2_nonzero:
        b2b_d = nc.dram_tensor("b2b", [P, D], f32, kind="ExternalInput")
    out_d = nc.dram_tensor("out", [NBC * P, D], f32, kind="ExternalOutput")

    with tile.TileContext(nc) as tc:
        with tc.tile_pool(name="meta", bufs=1) as meta, \
             tc.tile_pool(name="gat", bufs=2) as gat, \
             tc.tile_pool(name="msk", bufs=10) as msk, \
             tc.tile_pool(name="eptp", bufs=3) as eptp, \
             tc.tile_pool(name="ps_agg", bufs=2, space="PSUM") as ps_agg, \
             tc.tile_pool(name="ps_h", bufs=2, space="PSUM") as ps_h, \
             tc.tile_pool(name="ps_o", bufs=2, space="PSUM") as ps_o:

            ilo_sb = meta.tile([P, ST_lo], i16)
            nc.sync.dma_start(out=ilo_sb[:], in_=ilo_d.ap())
            ihi_sb = meta.tile([P, ST_hi], i16)
            nc.sync.dma_start(out=ihi_sb[:], in_=ihi_d.ap())
            dstl_sb = meta.tile([P, M], f32)
            nc.sync.dma_start(out=dstl_sb[:], in_=dstl_d.ap())
            se_sb = meta.tile([P, M], f32)
            nc.sync.dma_start(out=se_sb[:], in_=se_d.ap())
            iota_sb = meta.tile([P, P], f16)
            nc.sync.dma_start(out=iota_sb[:], in_=iota_d.ap())
            w1_sb = meta.tile([D, D], f16)
            nc.sync.dma_start(out=w1_sb[:], in_=w1_d.ap())
            w2_sb = meta.tile([D, D], f16)
            nc.sync.dma_start(out=w2_sb[:], in_=w2_d.ap())
            b1_sb = meta.tile([D, 1], f32)
            nc.sync.dma_start(out=b1_sb[:], in_=b1_d.ap())
            if b2_nonzero:
                b2b_sb = meta.tile([P, D], f32)
                nc.sync.dma_start(out=b2b_sb[:], in_=b2b_d.ap())
            if act_k:
                # negated dst-locals: ACT mask path computes |iota - dstl|
                # via activation bias, which adds (so bias = -dstl).
                ndstl_sb = meta.tile([P, M], f32)
                nc.vector.tensor_scalar(out=ndstl_sb[:], in0=dstl_sb[:],
                                        scalar1=-1.0, scalar2=None,
                                        op0=mybir.AluOpType.mult)

            def issue_gathers(gi, grp):
                SC_lo = sum(C_lo_s[b] for b in grp)
                SC_hi = sum(C_hi_s[b] for b in grp)
                xg_lo = gat.tile([P, SC_lo, D], f16, tag="xg_lo")
                o16 = lo_off16[grp[0]]
                nc.gpsimd.dma_gather(
                    out_ap=xg_lo[:], in_ap=xt.ap()[0:HALF, :],
                    idxs_ap=ilo_sb[:, o16:o16 + SC_lo * 8],
                    num_idxs=SC_lo * P, num_idxs_reg=SC_lo * P,
                    elem_size=D, single_packet=False,
                    queue_num=(2 * gi) % 3)
                xg_hi = gat.tile([P, SC_hi, D], f16, tag="xg_hi")
                o16 = hi_off16[grp[0]]
                nc.gpsimd.dma_gather(
                    out_ap=xg_hi[:], in_ap=xt.ap()[HALF:NPAD, :],
                    idxs_ap=ihi_sb[:, o16:o16 + SC_hi * 8],
                    num_idxs=SC_hi * P, num_idxs_reg=SC_hi * P,
                    elem_size=D, single_packet=False,
                    queue_num=(2 * gi + 1) % 3)
                return xg_lo, xg_hi

            for _ in range(repeats):
                tiles = issue_gathers(0, groups[0])
                for gi, grp in enumerate(groups):
                    xg_lo, xg_hi = tiles
                    if gi + 1 < len(groups):
                        tiles = issue_gathers(gi + 1, groups[gi + 1])
                    for b in grp:
                        C_lo, C_hi = C_lo_s[b], C_hi_s[b]
                        C = C_lo + C_hi
                        agg_ps = ps_agg.tile([D, P], f32, tag="agg")
                        parts = ((xg_lo, loc_lo[b], C_lo, col_lo[b]),
                                 (xg_hi, loc_hi[b], C_hi, col_hi[b]))
                        t = 0
                        for xg, loc, Cn, col0 in parts:
                            for u in range(Cn):
                                c = col0 + u
                                mask = msk.tile([P, P], f16, tag="mask")
                                if act_k and t % act_k == act_k - 1:
                                    ad = msk.tile([P, P], f16, tag="actm")
                                    nc.scalar.activation(
                                        ad[:], iota_sb[:],
                                        mybir.ActivationFunctionType.Abs,
                                        bias=ndstl_sb[:, c:c + 1], scale=1.0)
                                    rl = msk.tile([P, P], f16, tag="actr")
                                    nc.scalar.activation(
                                        rl[:], ad[:],
                                        mybir.ActivationFunctionType.Relu,
                                        bias=1.0, scale=-1.0)
                                    nc.scalar.activation(
                                        mask[:], rl[:],
                                        mybir.ActivationFunctionType.Identity,
                                        bias=0.0, scale=se_sb[:, c:c + 1])
                                elif pool_k and t % pool_k == pool_k - 2:
                                    nc.gpsimd.tensor_scalar(
                                        out=mask[:], in0=iota_sb[:],
                                        scalar1=dstl_sb[:, c:c + 1],
                                        scalar2=se_sb[:, c:c + 1],
                                        op0=mybir.AluOpType.is_equal,
                                        op1=mybir.AluOpType.mult)
                                else:
                                    nc.vector.tensor_scalar(
                                        out=mask[:], in0=iota_sb[:],
                                        scalar1=dstl_sb[:, c:c + 1],
                                        scalar2=se_sb[:, c:c + 1],
                                        op0=mybir.AluOpType.is_equal,
                                        op1=mybir.AluOpType.mult)
                                nc.tensor.matmul(out=agg_ps[:],
                                                 lhsT=xg[:, loc + u, :],
                                                 rhs=mask[:],
                                                 start=(t == 0),
                                                 stop=(t == C - 1))
                                t += 1
                        aggT_sb = eptp.tile([D, P], f16, tag="aggT")
                        nc.scalar.copy(aggT_sb[:], agg_ps[:])
                        h_ps = ps_h.tile([D, P], f32, tag="h")
                        nc.tensor.matmul(out=h_ps[:], lhsT=w1_sb[:],
                                         rhs=aggT_sb[:], start=True, stop=True)
                        hT_sb = eptp.tile([D, P], f16, tag="hT")
                        nc.scalar.activation(hT_sb[:], h_ps[:],
                                             mybir.ActivationFunctionType.Relu,
                                             bias=b1_sb[:, :1], scale=1.0)
                        o_ps = ps_o.tile([P, D], f32, tag="o")
                        nc.tensor.matmul(out=o_ps[:], lhsT=hT_sb[:],
                                         rhs=w2_sb[:], start=True, stop=True)
                        out_sb = eptp.tile([P, D], f32, tag="outsb")
                        if b2_nonzero:
                            nc.vector.tensor_tensor(out=out_sb[:], in0=o_ps[:],
                                                    in1=b2b_sb[:],
                                                    op=mybir.AluOpType.add)
                        else:
                            nc.scalar.copy(out_sb[:], o_ps[:])
                        nc.sync.dma_start(out=out_d.ap()[b * P:(b + 1) * P, :],
                                          in_=out_sb[:])
    nc.compile()
    return nc


def _make_in_maps(xpad, ilo16, ihi16, dstl_all, se_all,
                  W1, b1, W2, b2, b2_nonzero):
    iota = np.tile(np.arange(P, dtype=np.float16), (P, 1))
    in_maps = []
    for c in range(NCORES):
        m = {
            "xt": xpad,
            "ilo": ilo16[c],
            "ihi": ihi16[c],
            "dstl": dstl_all[c],
            "se": se_all[c],
            "iota": iota,
            "w1": np.asarray(W1, np.float16),
            "w2": np.asarray(W2, np.float16),
            "b1": np.asarray(b1, np.float32).reshape(D, 1),
        }
        if b2_nonzero:
            m["b2b"] = np.tile(np.asarray(b2, np.float32).reshape(1, D), (P, 1))
        in_maps.append(m)
    return in_maps


def kernel(x, src, dst, edge_weights, W1, b1, W2, b2):
    layout, xpad, ilo16, ihi16, dstl_all, se_all = \
        _host_pack(x, src, dst, edge_weights)
    b2_nonzero = bool(np.any(np.asarray(b2)))
    nc = _build_program(layout, b2_nonzero)
    in_maps = _make_in_maps(xpad, ilo16, ihi16, dstl_all, se_all,
                            W1, b1, W2, b2, b2_nonzero)
    res = run_bass_kernel_spmd(nc, in_maps, core_ids=list(range(NCORES)))
    out = np.concatenate([res.results[c]["out"] for c in range(NCORES)], axis=0)
    return out[:N_NODES].astype(np.float32)
